# revision 1
# baseline (speedup 1.0000x reference)
"""SwitchBack global-quantized MLP on 8 TRN2 NeuronCores.

Strategy: data-parallel over the 8192 token rows (1024 rows/core, zero
collectives).  Weights are globally int8-quantized on the host (static
prep, numerically identical to the reference: np.round == round-half-even)
and shipped pre-transposed in bf16 (int8 values are exact in bf16; all
matmul products accumulate exactly in fp32 PSUM).  Activations are
quantized on-device: rowwise absmax -> reciprocal -> magic-number
round-to-nearest-even -> bf16, DMA-xbar transpose to put the contraction
dim on partitions.
"""

import numpy as np
import ml_dtypes

import concourse.bass as bass
import concourse.mybir as mybir
import concourse.tile as tile
from concourse import bacc
from concourse.bass_utils import run_bass_kernel_spmd

Q = 127.0
MAGIC = 12582912.0  # 1.5 * 2**23: (v + MAGIC) - MAGIC == RNE-round(v) for |v| <= 2**22
P = 128
FD = 512  # matmul moving free dim == one PSUM bank of fp32

F32 = mybir.dt.float32
BF16 = mybir.dt.bfloat16


def build_program(NR, D, H, c1, c2, n_cores=8, gelu_mode="lut", reps=1,
                  bulk_eng="sync", tp_eng="sync"):
    """One-core SPMD program: NR token rows, x[NR,D] @ W1qT[D,H] -> gelu ->
    requant -> @ W2qT[H,D] -> out[NR,D].  c1/c2 = sW/(Q*Q) dequant consts."""
    MT, KD, NH, KH, ND = NR // P, D // P, H // FD, H // P, D // FD
    AF = mybir.ActivationFunctionType
    OP = mybir.AluOpType
    GELU_A = 0.044715
    GELU_2C = float(2.0 * np.sqrt(2.0 / np.pi))

    nc = bacc.Bacc("TRN2", target_bir_lowering=False, debug=False,
                   num_devices=n_cores)
    x_d = nc.dram_tensor("x", [NR, D], F32, kind="ExternalInput")
    w1_d = nc.dram_tensor("w1qt", [D, H], BF16, kind="ExternalInput")
    w2_d = nc.dram_tensor("w2qt", [H, D], BF16, kind="ExternalInput")
    b1_d = nc.dram_tensor("b1r", [P, H], F32, kind="ExternalInput")
    b2_d = nc.dram_tensor("b2r", [P, D], F32, kind="ExternalInput")
    out_d = nc.dram_tensor("out", [NR, D], F32, kind="ExternalOutput")
    x2_d = nc.dram_tensor("x2f", [NR, H], F32)  # gelu output, pre-requant

    with tile.TileContext(nc) as tc:
        with (
            tc.tile_pool(name="glob", bufs=1) as gp,
            tc.tile_pool(name="psum", bufs=8, space="PSUM") as pp,
        ):
            b2_sb = gp.tile([P, D], F32, tag="b2", name="b2_sb")
            getattr(nc, bulk_eng).dma_start(out=b2_sb, in_=b2_d[:, :])
            for rep in range(reps):
                # Persistent per-row scalars, one column per m-tile.
                ds1 = gp.tile([P, MT], F32, tag="ds1", name="ds1", bufs=2)
                rmax = gp.tile([P, MT], F32, tag="rmax", name="rmax", bufs=2)
                s2 = gp.tile([P, MT], F32, tag="s2", name="s2", bufs=2)
                ds2 = gp.tile([P, MT], F32, tag="ds2", name="ds2", bufs=2)
                nc.vector.memset(rmax, 0.0)

                # ---------------- phase A: quantize x, matmul1, gelu ----------
                with (
                    tc.tile_pool(name="pa", bufs=1) as pa,
                    tc.tile_pool(name="wa", bufs=16) as wa,
                    tc.tile_pool(name="ea", bufs=4) as ea,
                ):
                    b1_sb = pa.tile([P, H], F32, tag="b1", name="b1_sb")
                    getattr(nc, bulk_eng).dma_start(out=b1_sb, in_=b1_d[:, :])
                    x1T = [pa.tile([P, KD, P], BF16, tag=f"x1T{m}", name=f"x1T{m}")
                           for m in range(MT)]
                    for m in range(MT):
                        xt = ea.tile([P, D], F32, tag="xt", name=f"xt{m}", bufs=2)
                        getattr(nc, bulk_eng).dma_start(out=xt, in_=x_d[m * P:(m + 1) * P, :])
                        am = ea.tile([P, 1], F32, tag="am", name=f"am{m}")
                        nc.vector.tensor_reduce(am, xt, axis=mybir.AxisListType.X,
                                                op=OP.max, apply_absolute_value=True)
                        rr = ea.tile([P, 1], F32, tag="rr", name=f"rr{m}")
                        nc.vector.reciprocal(rr, am)
                        s1m = ea.tile([P, 1], F32, tag="s1m", name=f"s1m{m}")
                        nc.vector.tensor_scalar_mul(s1m, rr, Q)
                        nc.vector.tensor_scalar_mul(ds1[:, m:m + 1], am, c1)
                        tq = ea.tile([P, D], F32, tag="tq", name=f"tq{m}", bufs=2)
                        nc.scalar.activation(tq, xt, AF.Copy, bias=MAGIC, scale=s1m)
                        x1q = ea.tile([P, D], BF16, tag="x1q", name=f"x1q{m}", bufs=3)
                        nc.vector.tensor_scalar_sub(x1q, tq, MAGIC)
                        getattr(nc, tp_eng).dma_start_transpose(out=x1T[m][:, :, :],
                                                    in_=x1q)

                    for n in range(NH):
                        w1t = []
                        for k in range(KD):
                            w = wa.tile([P, FD], BF16, tag="w1t", name=f"w1t{n}_{k}")
                            getattr(nc, bulk_eng).dma_start(
                                out=w, in_=w1_d[k * P:(k + 1) * P,
                                                n * FD:(n + 1) * FD])
                            w1t.append(w)
                        pss = [pp.tile([P, FD], F32, tag="ps", name=f"psA{n}_{m}")
                               for m in range(MT)]
                        for k in range(KD):
                            for m in range(MT):
                                nc.tensor.matmul(pss[m],
                                                 x1T[m][:, k, :],
                                                 w1t[k],
                                                 start=(k == 0), stop=(k == KD - 1))
                        for m in range(MT):
                            t1 = ea.tile([P, FD], F32, tag="t1", name=f"t1_{n}_{m}")
                            nc.vector.scalar_tensor_tensor(
                                t1, pss[m], ds1[:, m:m + 1],
                                b1_sb[:, n * FD:(n + 1) * FD],
                                op0=OP.mult, op1=OP.add)
                            g = ea.tile([P, FD], F32, tag="g", name=f"g{n}_{m}")
                            if gelu_mode == "lut":
                                nc.scalar.activation(g, t1, AF.Gelu_apprx_tanh)
                            else:
                                # gelu_tanh(x) = x * sigmoid(2c * x * (1 + a*x^2))
                                sq = ea.tile([P, FD], F32, tag="sq", name=f"sq{n}_{m}")
                                nc.vector.tensor_tensor(sq, t1, t1, OP.mult)
                                nc.vector.tensor_scalar(sq, sq, GELU_A, 1.0,
                                                        op0=OP.mult, op1=OP.add)
                                nc.vector.tensor_tensor(sq, t1, sq, OP.mult)
                                sg = ea.tile([P, FD], F32, tag="sg", name=f"sg{n}_{m}")
                                nc.scalar.activation(sg, sq, AF.Sigmoid,
                                                     scale=GELU_2C)
                                nc.vector.tensor_tensor(g, t1, sg, OP.mult)
                            pm = ea.tile([P, 1], F32, tag="pm", name=f"pm{n}_{m}")
                            nc.vector.tensor_reduce(pm, g, axis=mybir.AxisListType.X,
                                                    op=OP.max,
                                                    apply_absolute_value=True)
                            nc.vector.tensor_tensor(rmax[:, m:m + 1],
                                                    rmax[:, m:m + 1], pm, OP.max)
                            getattr(nc, bulk_eng).dma_start(
                                out=x2_d[m * P:(m + 1) * P, n * FD:(n + 1) * FD],
                                in_=g)

                # ---------------- phase B: requantize X2; phase C: matmul2 ----
                with (
                    tc.tile_pool(name="pc", bufs=1) as pc,
                    tc.tile_pool(name="wc", bufs=16) as wc,
                    tc.tile_pool(name="eb", bufs=4) as eb,
                ):
                    rr2 = eb.tile([P, MT], F32, tag="rr2", name="rr2")
                    nc.vector.reciprocal(rr2, rmax)
                    nc.vector.tensor_scalar_mul(s2, rr2, Q)
                    nc.vector.tensor_scalar_mul(ds2, rmax, c2)

                    JT = FD // P
                    x2T = {}
                    for nn in range(NH):
                        for mm in range(MT):
                            x2T[(nn, mm)] = pc.tile(
                                [P, JT, P], BF16, tag=f"x2T{nn}_{mm}",
                                name=f"x2T{nn}_{mm}")
                    for n in range(NH):
                        for m in range(MT):
                            xt2 = eb.tile([P, FD], F32, tag="xt2", name=f"xt2_{n}_{m}")
                            getattr(nc, bulk_eng).dma_start(
                                out=xt2, in_=x2_d[m * P:(m + 1) * P,
                                                  n * FD:(n + 1) * FD])
                            tq2 = eb.tile([P, FD], F32, tag="tq2", name=f"tq2_{n}_{m}")
                            nc.scalar.activation(tq2, xt2, AF.Copy, bias=MAGIC,
                                                 scale=s2[:, m:m + 1])
                            q2 = eb.tile([P, FD], BF16, tag="q2", name=f"q2_{n}_{m}")
                            nc.vector.tensor_scalar_sub(q2, tq2, MAGIC)
                            getattr(nc, tp_eng).dma_start_transpose(
                                out=x2T[(n, m)][:, :, :], in_=q2)

                    for d in range(ND):
                        pss2 = [pp.tile([P, FD], F32, tag="ps", name=f"psC{d}_{m}")
                                for m in range(MT)]
                        for k in range(KH):
                            w2t = wc.tile([P, FD], BF16, tag="w2t",
                                          name=f"w2t{d}_{k}")
                            getattr(nc, bulk_eng).dma_start(
                                out=w2t, in_=w2_d[k * P:(k + 1) * P,
                                                  d * FD:(d + 1) * FD])
                            for m in range(MT):
                                nc.tensor.matmul(pss2[m],
                                                 x2T[(k // JT, m)][:, k % JT, :],
                                                 w2t,
                                                 start=(k == 0), stop=(k == KH - 1))
                        for m in range(MT):
                            o = eb.tile([P, FD], F32, tag="o", name=f"o{d}_{m}")
                            nc.vector.scalar_tensor_tensor(
                                o, pss2[m], ds2[:, m:m + 1],
                                b2_sb[:, d * FD:(d + 1) * FD],
                                op0=OP.mult, op1=OP.add)
                            getattr(nc, bulk_eng).dma_start(
                                out=out_d[m * P:(m + 1) * P, d * FD:(d + 1) * FD],
                                in_=o)
    nc.compile()
    return nc


def _host_prep(x, W1, B1, W2, B2, n_cores=8):
    B, S, D = x.shape
    H = W1.shape[0]
    N = B * S
    NR = N // n_cores
    X = np.ascontiguousarray(x.reshape(N, D))

    def quant_global_T(w):
        # match jnp: absmax in f32, scale = f32(127)/absmax, round-half-even
        am = np.float32(np.max(np.abs(w)))
        scale = np.float32(Q) / am
        q = np.round(w.astype(np.float32) * scale)
        return np.ascontiguousarray(q.T).astype(ml_dtypes.bfloat16), am

    W1qT, sW1 = quant_global_T(W1)  # [D, H]
    W2qT, sW2 = quant_global_T(W2)  # [H, D]
    c1 = float(sW1) / (Q * Q)
    c2 = float(sW2) / (Q * Q)
    b1r = np.ascontiguousarray(np.broadcast_to(B1.astype(np.float32), (P, H)))
    b2r = np.ascontiguousarray(np.broadcast_to(B2.astype(np.float32), (P, D)))

    in_maps = [
        {"x": X[i * NR:(i + 1) * NR], "w1qt": W1qT, "w2qt": W2qT,
         "b1r": b1r, "b2r": b2r}
        for i in range(n_cores)
    ]
    return in_maps, NR, D, H, c1, c2


def _run_sharded(nc, in_maps, n_cores, bench_iters=0):
    """Mirror bass2jax.run_bass_via_pjrt's multi-core path, with an optional
    steady-state timing loop over device-resident inputs."""
    import time

    import jax
    from jax.sharding import Mesh, NamedSharding, PartitionSpec
    from jax.experimental.shard_map import shard_map
    import concourse.mybir as mybir_
    from concourse import bass2jax

    bass2jax.install_neuronx_cc_hook()

    partition_name = (nc.partition_id_tensor.name
                      if nc.partition_id_tensor else None)
    in_names, out_names, out_avals, zero_outs = [], [], [], []
    for alloc in nc.m.functions[0].allocations:
        if not isinstance(alloc, mybir_.MemoryLocationSet):
            continue
        name = alloc.memorylocations[0].name
        if alloc.kind == "ExternalInput":
            if name != partition_name:
                in_names.append(name)
        elif alloc.kind == "ExternalOutput":
            out_names.append(name)
            shape = tuple(alloc.tensor_shape)
            dtype = mybir_.dt.np(alloc.dtype)
            out_avals.append(jax.core.ShapedArray(shape, dtype))
            zero_outs.append(np.zeros(shape, dtype))
    n_params = len(in_names)
    n_outs = len(out_avals)
    in_names = in_names + out_names
    if partition_name is not None:
        in_names.append(partition_name)
    donate = tuple(range(n_params, n_params + n_outs))

    def _body(*args):
        operands = list(args)
        if partition_name is not None:
            operands.append(bass2jax.partition_id_tensor())
        return tuple(bass2jax._bass_exec_p.bind(
            *operands,
            out_avals=tuple(out_avals),
            in_names=tuple(in_names),
            out_names=tuple(out_names),
            lowering_input_output_aliases=(),
            sim_require_finite=True,
            sim_require_nnan=True,
            nc=nc,
        ))

    devices = jax.devices()[:n_cores]
    mesh = Mesh(np.asarray(devices), ("core",))
    spec = NamedSharding(mesh, PartitionSpec("core"))
    sharded = jax.jit(
        shard_map(_body, mesh=mesh,
                  in_specs=(PartitionSpec("core"),) * (n_params + n_outs),
                  out_specs=(PartitionSpec("core"),) * n_outs,
                  check_rep=False),
        donate_argnums=donate, keep_unused=True)

    concat_in = [
        np.concatenate([np.asarray(in_maps[c][name]) for c in range(n_cores)],
                       axis=0)
        for name in in_names[:n_params]
    ]
    dev_in = [jax.device_put(a, spec) for a in concat_in]
    big_zeros = [np.zeros((n_cores * z.shape[0], *z.shape[1:]), z.dtype)
                 for z in zero_outs]

    def fresh_zeros():
        return [jax.device_put(z, spec) for z in big_zeros]

    out_arrs = sharded(*dev_in, *fresh_zeros())
    jax.block_until_ready(out_arrs)

    per_iter_s = None
    if bench_iters > 1:
        zero_sets = [fresh_zeros() for _ in range(bench_iters)]
        jax.block_until_ready(zero_sets)
        t0 = time.perf_counter()
        last = None
        for k in range(bench_iters):
            last = sharded(*dev_in, *zero_sets[k])
        jax.block_until_ready(last)
        per_iter_s = (time.perf_counter() - t0) / bench_iters

    results = [
        {name: np.asarray(out_arrs[i]).reshape(n_cores, *out_avals[i].shape)[c]
         for i, name in enumerate(out_names)}
        for c in range(n_cores)
    ]
    return results, per_iter_s


def kernel_with_results(x, W1, B1, W2, B2, bench_iters=0, reps=1,
                        gelu_mode="lut"):
    n_cores = 8
    in_maps, NR, D, H, c1, c2 = _host_prep(x, W1, B1, W2, B2, n_cores)
    nc = build_program(NR, D, H, c1, c2, n_cores, gelu_mode=gelu_mode,
                       reps=reps)
    results, per_iter_s = _run_sharded(nc, in_maps, n_cores, bench_iters)
    out = np.concatenate([r["out"] for r in results], axis=0)
    return out.reshape(x.shape).astype(np.float32), per_iter_s


def kernel(x, W1, B1, W2, B2):
    return kernel_with_results(x, W1, B1, W2, B2)[0]



# revision 19
# speedup vs baseline: 1.8454x; 1.8454x over previous
"""SwitchBack global-quantized MLP on 8 TRN2 NeuronCores.

Strategy: data-parallel over the 8192 token rows (1024 rows/core, zero
collectives).  Weights are globally int8-quantized on the host and shipped
pre-tiled in bf16 (int8 values exact in bf16).

Single-pass fused pipeline per core (no DRAM round trip for the hidden
activation, no 16MB transpose):
  - quantize x rowwise, fold the per-token dequant scale ds1 into the
    activation values, DMA-xbar-transpose into x1T [D-part, tok].
  - matmul1 runs WEIGHT-STATIONARY: out[h, tok] = W1tile.T @ x1T, so the
    PSUM output has hidden dim on partitions -> bias+gelu fuse into ONE
    scalar-engine op (bias is per-partition), result parked in SBUF bf16.
  - per-token absmax over H becomes a partition-dim reduction: running
    elementwise max across h-tiles, one small 0.25MB transpose, then a
    K=1 ones-matmul broadcasts the per-token scale back across partitions.
  - requantize in place (magic-number round), overlapped with matmul2.
  - matmul2 runs ACTIVATION-STATIONARY: lhsT = x2q tiles directly (already
    [h-part, tok]), rhs = W2 -> out[tok, d] in PSUM, dequant scale is
    per-partition again, direct DMA to DRAM output.
"""

import numpy as np
import ml_dtypes

import concourse.bass as bass
import concourse.mybir as mybir
import concourse.tile as tile
from concourse import bacc, bass_isa
from concourse.bass_utils import run_bass_kernel_spmd

Q = 127.0
MAGIC = 12582912.0  # 1.5 * 2**23: (v + MAGIC) - MAGIC == RNE-round(v), |v| <= 2**22
P = 128
FD = 512  # matmul moving free dim == one PSUM bank of fp32

F32 = mybir.dt.float32
BF16 = mybir.dt.bfloat16


def build_program(NR, D, H, c1, c2, n_cores=8, reps=1):
    """One-core SPMD program: NR token rows, out = dequant MLP per reference.
    c1/c2 = sW/(Q*Q) global-weight dequant constants."""
    MT = NR // P        # 8 token tiles
    KD = D // P         # 16 contraction tiles for matmul1
    HT = H // P         # 64 hidden tiles
    ND = D // FD        # 4 output column tiles
    HH = D // 2         # half-row for x prep (1024)
    AF = mybir.ActivationFunctionType
    OP = mybir.AluOpType

    nc = bacc.Bacc("TRN2", target_bir_lowering=False, debug=False,
                   num_devices=n_cores)
    x_d = nc.dram_tensor("x", [NR, D], F32, kind="ExternalInput")
    w1_d = nc.dram_tensor("w1til", [HT * P, KD * P], BF16, kind="ExternalInput")
    w2_d = nc.dram_tensor("w2til", [ND * HT * P, FD], BF16, kind="ExternalInput")
    b1_d = nc.dram_tensor("b1c", [P, HT], F32, kind="ExternalInput")
    b2_d = nc.dram_tensor("b2r", [P, D], BF16, kind="ExternalInput")
    out_d = nc.dram_tensor("out", [NR, D], F32, kind="ExternalOutput")

    with tile.TileContext(nc) as tc:
        with (
            tc.tile_pool(name="glob", bufs=1) as gp,
            tc.tile_pool(name="work", bufs=1) as wp,
            tc.tile_pool(name="psum", bufs=8, space="PSUM") as pp,
        ):
            # ---- persistent constants ----
            b1col = gp.tile([P, HT], F32, tag="b1c", name="b1col")
            nc.sync.dma_start(out=b1col, in_=b1_d[:, :])
            b2row = gp.tile([P, D], BF16, tag="b2r", name="b2row")
            nc.sync.dma_start(out=b2row, in_=b2_d[:, :])

            for rep in range(reps):
                # ---- per-rep persistent tiles ----
                x1T = wp.tile([P, KD, NR], BF16, tag="x1T", name="x1T")
                x2 = [wp.tile([P, NR], BF16, tag=f"x2_{h}", name=f"x2_{h}")
                      for h in range(HT)]
                macc_hi = wp.tile([P, NR], BF16, tag="mhi", name="macc_hi")
                macc_lo = wp.tile([P, NR], BF16, tag="mlo", name="macc_lo")
                nc.vector.memset(macc_hi, 0.0)
                nc.vector.memset(macc_lo, 0.0)

                # ---------------- phase P: quantize x + fold ds1 + transpose
                for m in range(MT):
                    amh = wp.tile([P, 2], F32, tag="amh", name=f"amh{m}", bufs=2)
                    xh = []
                    for c in range(2):
                        xt = wp.tile([P, HH], F32, tag="xt", name=f"xt{m}_{c}",
                                     bufs=2)
                        nc.sync.dma_start(
                            out=xt, in_=x_d[m * P:(m + 1) * P,
                                            c * HH:(c + 1) * HH])
                        nc.vector.tensor_reduce(
                            amh[:, c:c + 1], xt, axis=mybir.AxisListType.X,
                            op=OP.max, apply_absolute_value=True)
                        xh.append(xt)
                    am = wp.tile([P, 1], F32, tag="am", name=f"am{m}", bufs=2)
                    nc.vector.tensor_reduce(am, amh, axis=mybir.AxisListType.X,
                                            op=OP.max)
                    rr = wp.tile([P, 1], F32, tag="rr", name=f"rr{m}", bufs=2)
                    nc.vector.reciprocal(rr, am)
                    s1 = wp.tile([P, 1], F32, tag="s1", name=f"s1_{m}", bufs=2)
                    nc.vector.tensor_scalar_mul(s1, rr, Q)
                    ds1 = wp.tile([P, 1], F32, tag="ds1", name=f"ds1_{m}", bufs=2)
                    nc.vector.tensor_scalar_mul(ds1, am, c1)
                    for c in range(2):
                        # in-place: xt <- xt*s1 + MAGIC  (f32 RNE to integer)
                        nc.vector.tensor_scalar(
                            out=xh[c], in0=xh[c], scalar1=s1, scalar2=MAGIC,
                            op0=OP.mult, op1=OP.add)
                        x1s = wp.tile([P, HH], BF16, tag="x1s",
                                      name=f"x1s{m}_{c}", bufs=2)
                        # x1s <- (xt - MAGIC) * ds1   (int8 value * dequant scale)
                        nc.vector.tensor_scalar(
                            out=x1s, in0=xh[c], scalar1=MAGIC, scalar2=ds1,
                            op0=OP.subtract, op1=OP.mult)
                        nc.sync.dma_start_transpose(
                            out=x1T[:, c * (KD // 2):(c + 1) * (KD // 2),
                                    m * P:(m + 1) * P],
                            in_=x1s)

                # ---------------- phase A: matmul1 (weight-stationary) + gelu
                for h in range(HT):
                    w1t = wp.tile([P, KD, P], BF16, tag="w1t", name=f"w1t{h}",
                                  bufs=2)
                    nc.sync.dma_start(out=w1t[:, :, :],
                                      in_=w1_d[h * P:(h + 1) * P, :])
                    ps = [pp.tile([P, FD], F32, tag="ps", name=f"psA{h}_{c}")
                          for c in range(2)]
                    for k in range(KD):
                        for c in range(2):
                            nc.tensor.matmul(ps[c], w1t[:, k, :],
                                             x1T[:, k, c * FD:(c + 1) * FD],
                                             start=(k == 0), stop=(k == KD - 1))
                    g = x2[h]
                    for c in range(2):
                        nc.scalar.activation(g[:, c * FD:(c + 1) * FD], ps[c],
                                             AF.Gelu_apprx_tanh,
                                             bias=b1col[:, h:h + 1])
                    nc.vector.tensor_tensor(macc_hi, macc_hi, g, OP.max)
                    nc.vector.tensor_tensor(macc_lo, macc_lo, g, OP.min)

                # ---------------- dance: per-token absmax -> scales
                # one gpsimd partition-all-reduce gives the per-token row max
                # ALREADY broadcast across all partitions
                # in-place combine: macc_hi = max(-macc_lo, macc_hi)
                nc.vector.scalar_tensor_tensor(macc_hi, macc_lo, -1.0, macc_hi,
                                               op0=OP.mult, op1=OP.max)
                s2b = wp.tile([P, NR], BF16, tag="s2b", name="s2b")
                nc.gpsimd.partition_all_reduce(s2b, macc_hi, P,
                                               bass_isa.ReduceOp.absmax)
                rrb = wp.tile([P, NR], F32, tag="xt", name="rrb", bufs=2)
                nc.vector.reciprocal(rrb, s2b)
                nc.vector.tensor_scalar_mul(s2b, rrb, Q)  # s2b = Q/rowmax (bf16)
                # consistent dequant scale per token, in token-partition layout:
                # ds2[p, m] = Q*c2 / s2(token m*128+p)
                s2bT = wp.tile([P, MT, P], BF16, tag="tmp", name="s2bT")
                nc.sync.dma_start_transpose(out=s2bT[:, :, :], in_=s2b)
                rmax_t = wp.tile([P, MT], F32, tag="rmax", name="rmax_t")
                nc.vector.reciprocal(rmax_t, s2bT[:, :, 0])
                ds2col = wp.tile([P, MT], F32, tag="ds2c", name="ds2col")
                nc.vector.tensor_scalar_mul(ds2col, rmax_t, Q * c2)

                # ---------------- phase B: requantize x2 in place
                # all-bf16 operands -> DVE 2x/4x modes keep pace with matmul2
                for h in range(HT):
                    tmp = wp.tile([P, NR], BF16, tag="tmp", name=f"tmp{h}",
                                  bufs=1)
                    nc.vector.tensor_tensor(tmp, x2[h], s2b, OP.mult)
                    nc.vector.tensor_scalar(
                        out=x2[h], in0=tmp, scalar1=MAGIC, scalar2=MAGIC,
                        op0=OP.add, op1=OP.subtract)

                # ---------------- phase C: matmul2 (activation-stationary)
                for d in range(ND):
                    ps2 = [pp.tile([P, FD], F32, tag="ps", name=f"ps2_{d}_{m}")
                           for m in range(MT)]
                    for k in range(HT):
                        w2t = wp.tile([P, FD], BF16, tag="w2t",
                                      name=f"w2t{d}_{k}", bufs=6)
                        nc.sync.dma_start(
                            out=w2t,
                            in_=w2_d[(d * HT + k) * P:(d * HT + k + 1) * P, :])
                        for m in range(MT):
                            nc.tensor.matmul(ps2[m],
                                             x2[k][:, m * P:(m + 1) * P],
                                             w2t,
                                             start=(k == 0), stop=(k == HT - 1))
                    for m in range(MT):
                        o = wp.tile([P, FD], F32, tag="o", name=f"o{d}_{m}",
                                    bufs=2)
                        if m % 2 == 0:
                            nc.vector.scalar_tensor_tensor(
                                o, ps2[m], ds2col[:, m:m + 1],
                                b2row[:, d * FD:(d + 1) * FD],
                                op0=OP.mult, op1=OP.add)
                        else:
                            # drain PSUM via ACT (per-partition scale), add
                            # bias on the idle gpsimd engine in SBUF
                            nc.scalar.activation(o, ps2[m], AF.Copy,
                                                 scale=ds2col[:, m:m + 1])
                            nc.gpsimd.tensor_tensor(
                                o, o, b2row[:, d * FD:(d + 1) * FD], OP.add)
                        nc.sync.dma_start(
                            out=out_d[m * P:(m + 1) * P, d * FD:(d + 1) * FD],
                            in_=o)
    nc.compile()
    return nc


def _host_prep(x, W1, B1, W2, B2, n_cores=8):
    B, S, D = x.shape
    H = W1.shape[0]
    N = B * S
    NR = N // n_cores
    HT = H // P
    KD = D // P
    ND = D // FD
    X = np.ascontiguousarray(x.reshape(N, D))

    def quant_global(w):
        # match jnp: absmax in f32, scale = f32(127)/absmax, round-half-even
        am = np.float32(np.max(np.abs(w)))
        scale = np.float32(Q) / am
        q = np.round(w.astype(np.float32) * scale)
        return q, am

    W1q, sW1 = quant_global(W1)   # [H, D]
    W2q, sW2 = quant_global(W2)   # [D, H]
    c1 = float(sW1) / (Q * Q)
    c2 = float(sW2) / (Q * Q)

    W1qT = W1q.T                  # [D, H]
    # w1til[h*P+p, k*P+q] = W1qT[k*P+p, h*P+q]
    w1til = np.ascontiguousarray(
        W1qT.reshape(KD, P, HT, P).transpose(2, 1, 0, 3).reshape(HT * P, KD * P)
    ).astype(ml_dtypes.bfloat16)
    W2qT = W2q.T                  # [H, D]
    # w2til[(d*HT+k)*P+p, q] = W2qT[k*P+p, d*FD+q]
    w2til = np.ascontiguousarray(
        W2qT.reshape(HT, P, ND, FD).transpose(2, 0, 1, 3).reshape(ND * HT * P, FD)
    ).astype(ml_dtypes.bfloat16)

    b1c = np.ascontiguousarray(B1.astype(np.float32).reshape(HT, P).T)  # [P, HT]
    b2r = np.ascontiguousarray(
        np.broadcast_to(B2.astype(np.float32), (P, D))).astype(ml_dtypes.bfloat16)

    in_maps = [
        {"x": X[i * NR:(i + 1) * NR], "w1til": w1til, "w2til": w2til,
         "b1c": b1c, "b2r": b2r}
        for i in range(n_cores)
    ]
    return in_maps, NR, D, H, c1, c2


def _run_sharded(nc, in_maps, n_cores, bench_iters=0):
    """Mirror bass2jax.run_bass_via_pjrt's multi-core path, with an optional
    steady-state timing loop over device-resident inputs."""
    import time

    import jax
    from jax.sharding import Mesh, NamedSharding, PartitionSpec
    from jax.experimental.shard_map import shard_map
    import concourse.mybir as mybir_
    from concourse import bass2jax

    bass2jax.install_neuronx_cc_hook()

    partition_name = (nc.partition_id_tensor.name
                      if nc.partition_id_tensor else None)
    in_names, out_names, out_avals, zero_outs = [], [], [], []
    for alloc in nc.m.functions[0].allocations:
        if not isinstance(alloc, mybir_.MemoryLocationSet):
            continue
        name = alloc.memorylocations[0].name
        if alloc.kind == "ExternalInput":
            if name != partition_name:
                in_names.append(name)
        elif alloc.kind == "ExternalOutput":
            out_names.append(name)
            shape = tuple(alloc.tensor_shape)
            dtype = mybir_.dt.np(alloc.dtype)
            out_avals.append(jax.core.ShapedArray(shape, dtype))
            zero_outs.append(np.zeros(shape, dtype))
    n_params = len(in_names)
    n_outs = len(out_avals)
    in_names = in_names + out_names
    if partition_name is not None:
        in_names.append(partition_name)
    donate = tuple(range(n_params, n_params + n_outs))

    def _body(*args):
        operands = list(args)
        if partition_name is not None:
            operands.append(bass2jax.partition_id_tensor())
        return tuple(bass2jax._bass_exec_p.bind(
            *operands,
            out_avals=tuple(out_avals),
            in_names=tuple(in_names),
            out_names=tuple(out_names),
            lowering_input_output_aliases=(),
            sim_require_finite=True,
            sim_require_nnan=True,
            nc=nc,
        ))

    devices = jax.devices()[:n_cores]
    mesh = Mesh(np.asarray(devices), ("core",))
    spec = NamedSharding(mesh, PartitionSpec("core"))
    sharded = jax.jit(
        shard_map(_body, mesh=mesh,
                  in_specs=(PartitionSpec("core"),) * (n_params + n_outs),
                  out_specs=(PartitionSpec("core"),) * n_outs,
                  check_rep=False),
        donate_argnums=donate, keep_unused=True)

    concat_in = [
        np.concatenate([np.asarray(in_maps[c][name]) for c in range(n_cores)],
                       axis=0)
        for name in in_names[:n_params]
    ]
    dev_in = [jax.device_put(a, spec) for a in concat_in]
    big_zeros = [np.zeros((n_cores * z.shape[0], *z.shape[1:]), z.dtype)
                 for z in zero_outs]

    def fresh_zeros():
        return [jax.device_put(z, spec) for z in big_zeros]

    out_arrs = sharded(*dev_in, *fresh_zeros())
    jax.block_until_ready(out_arrs)

    per_iter_s = None
    if bench_iters > 1:
        zero_sets = [fresh_zeros() for _ in range(bench_iters)]
        jax.block_until_ready(zero_sets)
        t0 = time.perf_counter()
        last = None
        for k in range(bench_iters):
            last = sharded(*dev_in, *zero_sets[k])
        jax.block_until_ready(last)
        per_iter_s = (time.perf_counter() - t0) / bench_iters

    results = [
        {name: np.asarray(out_arrs[i]).reshape(n_cores, *out_avals[i].shape)[c]
         for i, name in enumerate(out_names)}
        for c in range(n_cores)
    ]
    return results, per_iter_s


def kernel_with_results(x, W1, B1, W2, B2, bench_iters=0, reps=1):
    n_cores = 8
    in_maps, NR, D, H, c1, c2 = _host_prep(x, W1, B1, W2, B2, n_cores)
    nc = build_program(NR, D, H, c1, c2, n_cores, reps=reps)
    results, per_iter_s = _run_sharded(nc, in_maps, n_cores, bench_iters)
    out = np.concatenate([r["out"] for r in results], axis=0)
    return out.reshape(x.shape).astype(np.float32), per_iter_s


def kernel(x, W1, B1, W2, B2):
    return kernel_with_results(x, W1, B1, W2, B2)[0]
